# revision 38
# baseline (speedup 1.0000x reference)
"""DeformConv2d (DCNv2) Trainium2 Bass kernel.

Problem: N=4, C_IN=C_OUT=64, H=W=128, 3x3 taps, stride=1, pad=1, dil=1,
modulated deformable conv (torchvision semantics).

Sharding: 8 cores; core = (image n = core//2, row-half = core%2).
Each core computes out[n, :, i0:i0+64, :] from the full image x[n].

Host prep (pure input layout/index math, no conv arithmetic):
  - R4 table: R4[y, x, c, corner] bf16 = the 4 bilinear corner pixels of
    anchor (y,x), corner-innermost.  One 512B gather descriptor fetches all
    four corners of one (tap, out-pixel) sample.
  - idxs: int16 gather indices  base(k,i) + j + floor(dy)*PW + floor(dx),
    wrapped in 16 partitions (j = 16*jw + p%16) and replicated across the
    8 partition groups, as SWDGE expects.
  - w4: per-sample bilinear corner weights (frac parts, modulation mask
    folded in) in bf16, laid out so every DVE combine operand has a packed
    innermost dim (2x_1p fast path).

Device pipeline per 16-row block x 9 taps:
  1. Pool/SWDGE dma_gather (1024 descriptors/call, 4 queues round-robin)
     from R4 into g[j, (i, c, corner)].
  2. DVE: p4 = g * w4 (bf16), s2 = x-corner pairs summed; GpSimd: s =
     y-corner pairs summed.
  3. PE: per-row transposes [128j, 64c] -> [64c, 128j] (bf16), taps paired
     on partition halves for full 128-deep contraction; 5 accumulating
     matmul groups per block into PSUM.
"""
import sys
import os

_TRN_REPO = "/opt/trn_rl_repo"
if _TRN_REPO not in sys.path:
    sys.path.insert(0, _TRN_REPO)

import numpy as np
import ml_dtypes

import concourse.bass as bass
import concourse.bacc as bacc
import concourse.tile as tile
import concourse.mybir as mybir
from concourse.bass_utils import run_bass_kernel_spmd
from contextlib import ExitStack

F32 = mybir.dt.float32
BF16 = mybir.dt.bfloat16
I16 = mybir.dt.int16
ALU = mybir.AluOpType
NPBF16 = ml_dtypes.bfloat16

N, C, H, W = 4, 64, 128, 128
K2 = 9
PAD = 16                    # coordinate padding on each side
PH = H + 2 * PAD            # 160
PW = W + 2 * PAD            # 160
NENT = PH * PW              # 25600 R4 entries (64ch x 4 corners each)
HI = 64                     # rows per core
R = 16                      # rows per block
NBLK = HI // R              # 4
CLAMP = 11                  # |floor(offset)| clamp (pad-region safe)

_CACHED = {}


def build_nc():
    nc = bacc.Bacc(trn_type="TRN2", debug=False, num_swdge_queues=4)

    r4_d = nc.dram_tensor("r4", [NENT * 4 * C], BF16, kind="ExternalInput")
    idxs_d = nc.dram_tensor("idxs", [128, K2 * HI * 8], I16, kind="ExternalInput").ap()
    w4_d = nc.dram_tensor("w4", [128, K2 * HI * 4], BF16, kind="ExternalInput").ap()
    wky_d = nc.dram_tensor("wky", [128, K2 * 64], BF16, kind="ExternalInput").ap()
    ident_d = nc.dram_tensor("ident", [128, 128], BF16, kind="ExternalInput").ap()
    out_d = nc.dram_tensor("out", [64, HI * W], F32, kind="ExternalOutput").ap()

    # gather source: one 512B entry = 64ch x 4 bilinear corners bf16
    src_ap = bass.AP(r4_d, 0, [[4 * C, NENT - 1], [1, 4 * C]])

    with ExitStack() as ctx:
        tc = ctx.enter_context(tile.TileContext(nc))

        const = ctx.enter_context(tc.tile_pool(name="const", bufs=1))

        idxs = const.tile([128, K2 * HI * 8], I16)
        # k=0 chunk first so the first gathers can start before the bulk lands
        nc.sync.dma_start(idxs[:, 0:HI * 8], idxs_d[:, 0:HI * 8])
        nc.sync.dma_start(idxs[:, HI * 8:], idxs_d[:, HI * 8:])
        w4 = const.tile([128, K2 * HI * 4], BF16)
        nc.scalar.dma_start(w4[:], w4_d)
        wky = const.tile([128, K2 * 64], BF16)
        nc.scalar.dma_start(wky[:], wky_d)
        ident = const.tile([128, 128], BF16)
        nc.scalar.dma_start(ident[:], ident_d)

        gpool = ctx.enter_context(tc.tile_pool(name="g", bufs=8))
        p4pool = ctx.enter_context(tc.tile_pool(name="p4", bufs=3))
        s2pool = ctx.enter_context(tc.tile_pool(name="s2", bufs=3))
        stpool = ctx.enter_context(tc.tile_pool(name="st", bufs=2))
        obpool = ctx.enter_context(tc.tile_pool(name="ob", bufs=2))
        tpps = ctx.enter_context(tc.tile_pool(name="tp", bufs=2, space="PSUM"))
        outps = ctx.enter_context(tc.tile_pool(name="ops", bufs=1, space="PSUM"))

        idxs4 = idxs[:].rearrange("p (k i jw) -> p k i jw", k=K2, i=HI, jw=8)
        w4_5 = w4[:].rearrange("p (k i xc yc) -> p k i xc yc",
                               k=K2, i=HI, xc=2, yc=2)

        qn = [0]
        for b in range(NBLK):
            out_ps = outps.tile([64, R * W], F32)
            for k in range(K2):
                g = gpool.tile([128, R * 4 * C], BF16)
                # SWDGE ring holds 1024 descriptors -> 8 rows (1024 idxs)
                # per call, round-robined over 4 queues.  Deep gpool
                # buffering lets the gather stream run ahead of the combine.
                gv = g[:].rearrange("p (s e) -> p s e", s=R, e=4 * C)
                for sub in range(2):
                    nidx = 8 * 128
                    q = qn[0] % 4
                    qn[0] += 1
                    nc.gpsimd.dma_gather(
                        gv[:, sub * 8:(sub + 1) * 8, :],
                        src_ap,
                        idxs4[:, k, b * R + sub * 8:b * R + (sub + 1) * 8, :],
                        nidx,
                        nidx,
                        elem_size=4 * C,
                        elem_step=4 * C,
                        queue_num=q,
                    )
                # weighted corners (bf16).  g layout per row: (c, cr) with
                # the 4 corners innermost -> every operand's innermost dim
                # is packed (w4 broadcasts over c on a middle dim), so these
                # run in the DVE 2x_1p mode.
                p4 = p4pool.tile([128, R * 4 * C], BF16)
                wsl = w4_5[:, k, b * R:(b + 1) * R, :, :]
                w_b = bass.AP(
                    wsl.tensor, wsl.offset,
                    [wsl.ap[0], [4, R], [0, C], [1, 4]],
                )
                nc.vector.tensor_tensor(
                    p4[:].rearrange("p (i c cr) -> p i c cr", i=R, c=C, cr=4),
                    g[:].rearrange("p (i c cr) -> p i c cr", i=R, c=C, cr=4),
                    w_b, ALU.mult)
                # sum x-corners (cr = (xc, yc)); the y-corner sum is folded
                # into the matmul contraction (c,yc depth 128, weights
                # replicated per yc)
                s2 = s2pool.tile([128, R * C * 2], BF16)
                p4v = p4[:].rearrange("p (i c xc yc) -> p i c xc yc",
                                      i=R, c=C, xc=2, yc=2)
                nc.vector.tensor_tensor(
                    s2[:].rearrange("p (i c yc) -> p i c yc", i=R, c=C, yc=2),
                    p4v[:, :, :, 0, :], p4v[:, :, :, 1, :], ALU.add)
                # transpose [128j, (c,yc)=128] -> [(c,yc), 128j] per row
                s2v = s2[:].rearrange("p (i cy) -> p i cy", i=R, cy=2 * C)
                st2 = stpool.tile([128, R * 128], BF16)
                for h in range(R // 8):
                    tp = tpps.tile([128, 8 * 128], BF16)
                    for i2 in range(8):
                        i = h * 8 + i2
                        nc.tensor.transpose(
                            tp[:, i2 * 128:(i2 + 1) * 128],
                            s2v[:, i, :], ident[:])
                    nc.scalar.copy(
                        st2[:, h * 8 * 128:(h + 1) * 8 * 128], tp[:])
                for c4 in range(R * W // 512):
                    nc.tensor.matmul(
                        out_ps[:, c4 * 512:(c4 + 1) * 512],
                        wky[:, k * 64:(k + 1) * 64],
                        st2[:, c4 * 512:(c4 + 1) * 512],
                        start=(k == 0), stop=(k == K2 - 1))
            ob = obpool.tile([64, R * W], F32)
            nc.scalar.copy(ob[:], out_ps[:])
            nc.sync.dma_start(out_d[:, b * R * W:(b + 1) * R * W], ob[:])

    if not nc.is_finalized():
        nc.finalize()
    return nc


def _prep_shared(x, weight):
    """Per-image R4 tables + weight tiles shared by both cores of an image."""
    # weight is [C_OUT, C_IN, KH, KW] -> [C_OUT, C_IN, K2]
    wf = weight.reshape(C, C, K2)
    # wky[c*2 + yc, k*64 + o] = W[o, c, k], replicated over yc so the matmul
    # contraction over (c, yc) sums the y-corner pair for free
    wt = wf.transpose(1, 2, 0)  # [c, k, o]
    wky = np.ascontiguousarray(
        np.repeat(wt[:, None, :, :], 2, axis=1).reshape(128, K2 * 64))
    r4s = []
    for n in range(N):
        xp = np.zeros((PH + 1, PW + 1, C), np.float32)
        xp[PAD:PAD + H, PAD:PAD + W, :] = x[n].transpose(1, 2, 0)
        xpb = xp.astype(NPBF16)
        # entry [c, cr], cr = (xc, yc): (y,x), (y+1,x), (y,x+1), (y+1,x+1)
        r4 = np.stack([xpb[:PH, :PW], xpb[1:PH + 1, :PW],
                       xpb[:PH, 1:PW + 1], xpb[1:PH + 1, 1:PW + 1]], axis=3)
        r4s.append(np.ascontiguousarray(r4).reshape(-1))
    return r4s, wky.astype(NPBF16)


def _prep_core(offset, mask, r4s, wky, core):
    n, half = core // 2, core % 2
    i0 = half * HI

    off = offset[n, :, i0:i0 + HI, :].reshape(K2, 2, HI, W)  # [k, (dy,dx), i, j]
    dy, dx = off[:, 0], off[:, 1]                            # [k, i, j]
    fy = np.floor(dy)
    fx = np.floor(dx)
    ry = (dy - fy).astype(np.float32)                        # frac in [0,1)
    rx = (dx - fx).astype(np.float32)
    fy = np.clip(fy, -CLAMP, CLAMP).astype(np.int32)
    fx = np.clip(fx, -CLAMP, CLAMP).astype(np.int32)

    k = np.arange(K2)
    ki, kj = k // 3, k % 3
    i = np.arange(HI)
    j = np.arange(W)
    # entry index = (y0)*PW + x0 of the 4-corner anchor
    idx = ((i0 + i[None, :, None] + ki[:, None, None] - 1 + PAD + fy) * PW
           + j[None, None, :] + kj[:, None, None] - 1 + PAD + fx)  # [k, i, j]
    assert idx.min() >= 0 and idx.max() < NENT - 1
    # wrap: partition p holds j = 16*jw + p%16, replicated over p//16
    u = np.arange(128) % 16
    idxw = idx.reshape(K2, HI, W // 16, 16)                  # [k, i, jw, u]
    idxs = np.ascontiguousarray(
        idxw.transpose(3, 0, 1, 2)[u].reshape(128, -1)).astype(np.int16)

    m = mask[n, :, i0:i0 + HI, :]                            # [k, i, j]
    # w4[j, (k, i, xc, yc)], cr=(xc,yc): x0y0, x0y1, x1y0, x1y1; mask folded
    wxm0 = (1.0 - rx) * m
    wxm1 = rx * m
    w4 = np.stack([wxm0 * (1.0 - ry), wxm0 * ry,
                   wxm1 * (1.0 - ry), wxm1 * ry], axis=-1)   # [k, i, j, 4]
    w4 = np.ascontiguousarray(
        w4.transpose(2, 0, 1, 3).reshape(128, -1)).astype(NPBF16)

    return {
        "r4": r4s[n],
        "idxs": idxs,
        "w4": w4,
        "wky": wky,
        "ident": np.eye(128, dtype=np.float32).astype(NPBF16),
    }


def _run(x, offset, mask, weight, trace=False, trace_kwargs=None):
    x = np.asarray(x, np.float32)
    offset = np.asarray(offset, np.float32)
    mask = np.asarray(mask, np.float32)
    weight = np.asarray(weight, np.float32)

    if "nc" not in _CACHED:
        _CACHED["nc"] = build_nc()
    nc = _CACHED["nc"]

    r4s, wky = _prep_shared(x, weight)
    in_maps = [
        _prep_core(offset, mask, r4s, wky, core) for core in range(8)
    ]
    if trace:
        res = run_bass_kernel_spmd(nc, in_maps, list(range(8)), trace=True,
                                   **(trace_kwargs or {}))
    else:
        res = run_bass_kernel_spmd(nc, in_maps, list(range(8)))
    out = np.empty((N, C, H, W), np.float32)
    for core in range(8):
        n, half = core // 2, core % 2
        out[n, :, half * HI:(half + 1) * HI, :] = (
            res.results[core]["out"].reshape(C, HI, W))
    return out, res


def kernel_traced(x, offset, mask, weight, trace=True, trace_kwargs=None):
    """Like kernel() but runs with NTFF tracing; returns (out, results)."""
    return _run(x, offset, mask, weight, trace=trace, trace_kwargs=trace_kwargs)


def kernel(x, offset, mask, weight):
    out, _ = _run(x, offset, mask, weight, trace=False)
    return out
